# revision 4
# baseline (speedup 1.0000x reference)
"""Conv2D (VALID, 3x3, NCHW) on 8 TRN2 NeuronCores via Bass/Tile.

Problem: x (32,128,56,56) f32, weight (256,128,3,3) f32, bias (256,) f32
         -> out (32,256,54,54) f32.

Strategy:
  - Data-parallel over batch: 4 images per core, 8 cores, no collectives.
  - Conv as implicit GEMM: for each kernel tap (kh,kw), a matmul with
    lhsT = weight[ci, co_tile] (K=Cin=128 partitions, M=128) and
    rhs  = shifted x window [Cin=128, 9 rows x 54 cols = 486], accumulating
    all 9 taps into one PSUM bank. 2 cout tiles x 6 row groups x 4 images
    = 48 accumulation groups x 9 matmuls per core.
  - Inputs cast to bf16 on host (PE runs at full rate); accumulation fp32.
  - Bias added during the PSUM->SBUF copy on DVE, then DMA out as f32.
"""

import numpy as np
import ml_dtypes

import concourse.bass as bass
import concourse.mybir as mybir
from concourse import bacc
import concourse.tile as tile
from concourse.bass_utils import run_bass_kernel_spmd

N, CIN, H, W = 32, 128, 56, 56
COUT, KH, KW = 256, 3, 3
HO, WO = H - KH + 1, W - KW + 1  # 54, 54
NCORES = 8
NPER = N // NCORES  # 4 images per core
CTILES = COUT // 128  # 2
RG = 9                # output rows per PSUM group
NG = HO // RG         # 6 row groups
NPIX = RG * WO        # 486 <= 512 (one fp32 PSUM bank)

BF16 = mybir.dt.bfloat16
F32 = mybir.dt.float32


def build_nc() -> bass.Bass:
    nc = bacc.Bacc(None)
    x_h = nc.dram_tensor("x", [NPER, CIN, H, W], BF16, kind="ExternalInput")
    w_h = nc.dram_tensor("w", [CIN, KH * KW * COUT], BF16, kind="ExternalInput")
    b_h = nc.dram_tensor("b", [COUT, 1], F32, kind="ExternalInput")
    o_h = nc.dram_tensor("out", [NPER, COUT, HO, WO], F32, kind="ExternalOutput")

    with tile.TileContext(nc) as tc:
        with (
            tc.tile_pool(name="wpool", bufs=1) as wpool,
            tc.tile_pool(name="bpool", bufs=1) as bpool,
            tc.tile_pool(name="xpool", bufs=4) as xpool,
            tc.tile_pool(name="opool", bufs=4) as opool,
            tc.tile_pool(name="psum", bufs=8, space="PSUM") as psum_pool,
        ):
            # PE warmup: small matmuls on memset data with no DMA deps keep
            # the PE busy during the input-DMA window so HAM un-throttles to
            # 2.4 GHz before the real matmuls begin.
            wu = wpool.tile([CIN, 64], BF16)
            nc.gpsimd.memset(wu[:], 0)
            wupt = psum_pool.tile([32, 64], F32, tag="pt")
            for _ in range(20):
                nc.tensor.matmul(wupt[:], wu[:, :32], wu[:, :64], start=True, stop=True)

            # Weight DMA chunked per tap so matmul(t) only waits on chunk t.
            wt = wpool.tile([CIN, KH * KW * COUT], BF16)
            for t in range(KH * KW):
                nc.sync.dma_start(
                    out=wt[:, t * COUT : (t + 1) * COUT],
                    in_=w_h[:, t * COUT : (t + 1) * COUT],
                )
            bias_t = bpool.tile([COUT // CTILES, CTILES], F32)
            for c in range(CTILES):
                nc.sync.dma_start(out=bias_t[:, c : c + 1], in_=b_h[c * 128 : (c + 1) * 128, :])

            for n in range(NPER):
                xt = xpool.tile([CIN, H, W], BF16, tag="xt")
                # Chunked by row-group region: group g's matmuls read rows
                # [g*RG, g*RG+RG+KH-2]; precise region deps let compute on
                # early rows start before the whole image lands.
                for g in range(NG):
                    r0 = g * RG
                    r1 = H if g == NG - 1 else (g + 1) * RG
                    nc.sync.dma_start(out=xt[:, r0:r1, :], in_=x_h[n, :, r0:r1, :])
                for c in range(CTILES):
                    for g in range(NG):
                        pt = psum_pool.tile([128, RG, WO], F32, tag="pt")
                        for t in range(KH * KW):
                            kh, kw = divmod(t, KW)
                            lhsT = wt[:, t * COUT + c * 128 : t * COUT + c * 128 + 128]
                            rhs = xt[:, g * RG + kh : g * RG + kh + RG, kw : kw + WO]
                            nc.tensor.matmul(
                                pt[:], lhsT, rhs,
                                start=(t == 0), stop=(t == KH * KW - 1),
                            )
                        ot = opool.tile([128, RG, WO], F32, tag="ot")
                        nc.vector.tensor_scalar_add(ot[:], pt[:], bias_t[:, c : c + 1])
                        nc.sync.dma_start(
                            out=o_h[n, c * 128 : (c + 1) * 128, g * RG : (g + 1) * RG, :],
                            in_=ot[:],
                        )
    nc.finalize()
    return nc


_NC_CACHE = None


def _get_nc():
    global _NC_CACHE
    if _NC_CACHE is None:
        _NC_CACHE = build_nc()
    return _NC_CACHE


def _prep_in_maps(x, weight, bias):
    bf16 = ml_dtypes.bfloat16
    # [ci, kh, kw, co] layout so lhsT slices are [ci, co_tile]
    w_t = np.ascontiguousarray(
        weight.astype(np.float32).transpose(1, 2, 3, 0).reshape(CIN, KH * KW * COUT)
    ).astype(bf16)
    b_t = np.ascontiguousarray(bias.astype(np.float32).reshape(COUT, 1))
    in_maps = []
    for i in range(NCORES):
        xs = np.ascontiguousarray(x[i * NPER : (i + 1) * NPER]).astype(bf16)
        in_maps.append({"x": xs, "w": w_t, "b": b_t})
    return in_maps


def run(x, weight, bias, trace=False):
    nc = _get_nc()
    in_maps = _prep_in_maps(x, weight, bias)
    res = run_bass_kernel_spmd(nc, in_maps, core_ids=list(range(NCORES)), trace=trace)
    out = np.concatenate([r["out"] for r in res.results], axis=0)
    return out, res


def kernel(x: np.ndarray, weight: np.ndarray, bias: np.ndarray) -> np.ndarray:
    out, _ = run(x, weight, bias, trace=False)
    return out.astype(np.float32)


if __name__ == "__main__":
    nc = build_nc()
    print("built ok:", len(nc.m.functions[0].blocks if hasattr(nc.m.functions[0], 'blocks') else []), "blocks")


# revision 6
# speedup vs baseline: 1.0574x; 1.0574x over previous
"""Conv2D (VALID, 3x3, NCHW) on 8 TRN2 NeuronCores via Bass/Tile.

Problem: x (32,128,56,56) f32, weight (256,128,3,3) f32, bias (256,) f32
         -> out (32,256,54,54) f32.

Strategy:
  - Data-parallel over batch: 4 images per core, 8 cores, no collectives.
  - Conv as implicit GEMM: for each kernel tap (kh,kw), a matmul with
    lhsT = weight[ci, co_tile] (K=Cin=128 partitions, M=128) and
    rhs  = shifted x window [Cin=128, 9 rows x 54 cols = 486], accumulating
    all 9 taps into one PSUM bank. 2 cout tiles x 6 row groups x 4 images
    = 48 accumulation groups x 9 matmuls per core.
  - Inputs cast to bf16 on host (PE runs at full rate); accumulation fp32.
  - Bias added during the PSUM->SBUF copy on DVE, then DMA out as f32.
"""

import numpy as np
import ml_dtypes

import concourse.bass as bass
import concourse.mybir as mybir
from concourse import bacc
import concourse.tile as tile
from concourse.bass_utils import run_bass_kernel_spmd

N, CIN, H, W = 32, 128, 56, 56
COUT, KH, KW = 256, 3, 3
HO, WO = H - KH + 1, W - KW + 1  # 54, 54
NCORES = 8
NPER = N // NCORES  # 4 images per core
CTILES = COUT // 128  # 2
RG = 9                # output rows per PSUM group
NG = HO // RG         # 6 row groups
NPIX = RG * WO        # 486 <= 512 (one fp32 PSUM bank)

BF16 = mybir.dt.bfloat16
F32 = mybir.dt.float32


def build_nc() -> bass.Bass:
    nc = bacc.Bacc(None)
    x_h = nc.dram_tensor("x", [NPER, CIN, H, W], BF16, kind="ExternalInput")
    w_h = nc.dram_tensor("w", [CIN, KH * KW * COUT], BF16, kind="ExternalInput")
    b_h = nc.dram_tensor("b", [COUT, 1], F32, kind="ExternalInput")
    o_h = nc.dram_tensor("out", [NPER, COUT, HO, WO], F32, kind="ExternalOutput")

    with tile.TileContext(nc) as tc:
        with (
            tc.tile_pool(name="wpool", bufs=1) as wpool,
            tc.tile_pool(name="bpool", bufs=1) as bpool,
            tc.tile_pool(name="xpool", bufs=4) as xpool,
            tc.tile_pool(name="opool", bufs=4) as opool,
            tc.tile_pool(name="psum", bufs=8, space="PSUM") as psum_pool,
        ):
            # PE warmup: small matmuls on memset data with no DMA deps keep
            # the PE busy during the input-DMA window so HAM un-throttles to
            # 2.4 GHz by the time the real matmuls begin.
            wu = wpool.tile([CIN, 64], BF16)
            nc.gpsimd.memset(wu[:], 0)
            wupt = psum_pool.tile([32, 64], F32, tag="pt")
            for _ in range(40):
                nc.tensor.matmul(wupt[:], wu[:, :32], wu[:, :64], start=True, stop=True)

            # Input DMAs all on the sync (SP) HWDGE ring: FIFO per ring, so
            # chunk order = arrival order. Sized so the first matmul group's
            # deps (w taps 0-4 + x0 rows 0-11) land as early as possible.
            wt = wpool.tile([CIN, KH * KW * COUT], BF16)
            half_w = (KH * KW // 2) * COUT  # taps 0-4
            nc.sync.dma_start(out=wt[:, :half_w], in_=w_h[:, :half_w])

            xts = []
            for n in range(NPER):
                xt = xpool.tile([CIN, H, W], BF16, tag="xt", name=f"xt{n}")
                xts.append(xt)
            nc.sync.dma_start(out=xts[0][:, 0:12, :], in_=x_h[0, :, 0:12, :])
            nc.sync.dma_start(out=wt[:, half_w:], in_=w_h[:, half_w:])
            nc.sync.dma_start(out=xts[0][:, 12:30, :], in_=x_h[0, :, 12:30, :])
            nc.sync.dma_start(out=xts[0][:, 30:56, :], in_=x_h[0, :, 30:56, :])
            bias_t = bpool.tile([COUT // CTILES, CTILES], F32)
            nc.sync.dma_start(
                out=bias_t[:], in_=b_h.rearrange("(c p) o -> p (c o)", p=128)
            )
            for n in range(1, NPER):
                nc.sync.dma_start(out=xts[n][:], in_=x_h[n])

            for n in range(NPER):
                xt = xts[n]
                for c in range(CTILES):
                    for g in range(NG):
                        pt = psum_pool.tile([128, RG, WO], F32, tag="pt")
                        for t in range(KH * KW):
                            kh, kw = divmod(t, KW)
                            lhsT = wt[:, t * COUT + c * 128 : t * COUT + c * 128 + 128]
                            rhs = xt[:, g * RG + kh : g * RG + kh + RG, kw : kw + WO]
                            nc.tensor.matmul(
                                pt[:], lhsT, rhs,
                                start=(t == 0), stop=(t == KH * KW - 1),
                            )
                        ot = opool.tile([128, RG, WO], F32, tag="ot")
                        nc.vector.tensor_scalar_add(ot[:], pt[:], bias_t[:, c : c + 1])
                        # Output DMAs ride the scalar (ACT) HWDGE ring so their
                        # sem waits never head-of-line block the input ring.
                        nc.scalar.dma_start(
                            out=o_h[n, c * 128 : (c + 1) * 128, g * RG : (g + 1) * RG, :],
                            in_=ot[:],
                        )
    nc.finalize()
    return nc


_NC_CACHE = None


def _get_nc():
    global _NC_CACHE
    if _NC_CACHE is None:
        _NC_CACHE = build_nc()
    return _NC_CACHE


def _prep_in_maps(x, weight, bias):
    bf16 = ml_dtypes.bfloat16
    # [ci, kh, kw, co] layout so lhsT slices are [ci, co_tile]
    w_t = np.ascontiguousarray(
        weight.astype(np.float32).transpose(1, 2, 3, 0).reshape(CIN, KH * KW * COUT)
    ).astype(bf16)
    b_t = np.ascontiguousarray(bias.astype(np.float32).reshape(COUT, 1))
    in_maps = []
    for i in range(NCORES):
        xs = np.ascontiguousarray(x[i * NPER : (i + 1) * NPER]).astype(bf16)
        in_maps.append({"x": xs, "w": w_t, "b": b_t})
    return in_maps


def run(x, weight, bias, trace=False):
    nc = _get_nc()
    in_maps = _prep_in_maps(x, weight, bias)
    res = run_bass_kernel_spmd(nc, in_maps, core_ids=list(range(NCORES)), trace=trace)
    out = np.concatenate([r["out"] for r in res.results], axis=0)
    return out, res


def kernel(x: np.ndarray, weight: np.ndarray, bias: np.ndarray) -> np.ndarray:
    out, _ = run(x, weight, bias, trace=False)
    return out.astype(np.float32)


if __name__ == "__main__":
    nc = build_nc()
    print("built ok:", len(nc.m.functions[0].blocks if hasattr(nc.m.functions[0], 'blocks') else []), "blocks")


# revision 7
# speedup vs baseline: 1.0701x; 1.0120x over previous
"""Conv2D (VALID, 3x3, NCHW) on 8 TRN2 NeuronCores via Bass/Tile.

Problem: x (32,128,56,56) f32, weight (256,128,3,3) f32, bias (256,) f32
         -> out (32,256,54,54) f32.

Strategy:
  - Data-parallel over batch: 4 images per core, 8 cores, no collectives.
  - Conv as implicit GEMM: for each kernel tap (kh,kw), a matmul with
    lhsT = weight[ci, co_tile] (K=Cin=128 partitions, M=128) and
    rhs  = shifted x window [Cin=128, 9 rows x 54 cols = 486], accumulating
    all 9 taps into one PSUM bank. 2 cout tiles x 6 row groups x 4 images
    = 48 accumulation groups x 9 matmuls per core.
  - Inputs cast to bf16 on host (PE runs at full rate); accumulation fp32.
  - Bias added during the PSUM->SBUF copy on DVE, then DMA out as f32.
"""

import numpy as np
import ml_dtypes

import concourse.bass as bass
import concourse.mybir as mybir
from concourse import bacc
import concourse.tile as tile
from concourse.bass_utils import run_bass_kernel_spmd

N, CIN, H, W = 32, 128, 56, 56
COUT, KH, KW = 256, 3, 3
HO, WO = H - KH + 1, W - KW + 1  # 54, 54
NCORES = 8
NPER = N // NCORES  # 4 images per core
CTILES = COUT // 128  # 2
RG = 9                # output rows per PSUM group
NG = HO // RG         # 6 row groups
NPIX = RG * WO        # 486 <= 512 (one fp32 PSUM bank)

BF16 = mybir.dt.bfloat16
F32 = mybir.dt.float32


def build_nc() -> bass.Bass:
    nc = bacc.Bacc(None)
    x_h = nc.dram_tensor("x", [NPER, CIN, H, W], BF16, kind="ExternalInput")
    w_h = nc.dram_tensor("w", [CIN, KH * KW * COUT], BF16, kind="ExternalInput")
    b_h = nc.dram_tensor("b", [COUT, 1], F32, kind="ExternalInput")
    o_h = nc.dram_tensor("out", [NPER, COUT, HO, WO], F32, kind="ExternalOutput")

    with tile.TileContext(nc) as tc:
        with (
            tc.tile_pool(name="wpool", bufs=1) as wpool,
            tc.tile_pool(name="bpool", bufs=1) as bpool,
            tc.tile_pool(name="xpool", bufs=4) as xpool,
            tc.tile_pool(name="opool", bufs=4) as opool,
            tc.tile_pool(name="psum", bufs=8, space="PSUM") as psum_pool,
        ):
            # PE warmup: small matmuls on memset data with no DMA deps keep
            # the PE busy during the input-DMA window so HAM un-throttles to
            # 2.4 GHz by the time the real matmuls begin.
            wu = wpool.tile([CIN, 64], BF16)
            nc.gpsimd.memset(wu[:], 0)
            wupt = psum_pool.tile([32, 64], F32, tag="pt")
            for _ in range(58):
                nc.tensor.matmul(wupt[:], wu[:, :32], wu[:, :64], start=True, stop=True)

            # Input DMAs split across the two HWDGE rings (each is FIFO):
            # weights on sync (SP) in parallel with x0 chunks on scalar (ACT),
            # so the first matmul group's deps land as early as possible.
            wt = wpool.tile([CIN, KH * KW * COUT], BF16)
            half_w = (KH * KW // 2) * COUT  # taps 0-4
            nc.sync.dma_start(out=wt[:, :half_w], in_=w_h[:, :half_w])
            nc.sync.dma_start(out=wt[:, half_w:], in_=w_h[:, half_w:])

            xts = []
            for n in range(NPER):
                xt = xpool.tile([CIN, H, W], BF16, tag="xt", name=f"xt{n}")
                xts.append(xt)
            nc.scalar.dma_start(out=xts[0][:, 0:12, :], in_=x_h[0, :, 0:12, :])
            nc.scalar.dma_start(out=xts[0][:, 12:30, :], in_=x_h[0, :, 12:30, :])
            nc.scalar.dma_start(out=xts[0][:, 30:56, :], in_=x_h[0, :, 30:56, :])
            bias_t = bpool.tile([COUT // CTILES, CTILES], F32)
            nc.sync.dma_start(
                out=bias_t[:], in_=b_h.rearrange("(c p) o -> p (c o)", p=128)
            )
            for n in range(1, NPER):
                nc.sync.dma_start(out=xts[n][:], in_=x_h[n])

            for n in range(NPER):
                xt = xts[n]
                for c in range(CTILES):
                    for g in range(NG):
                        pt = psum_pool.tile([128, RG, WO], F32, tag="pt")
                        for t in range(KH * KW):
                            kh, kw = divmod(t, KW)
                            lhsT = wt[:, t * COUT + c * 128 : t * COUT + c * 128 + 128]
                            rhs = xt[:, g * RG + kh : g * RG + kh + RG, kw : kw + WO]
                            nc.tensor.matmul(
                                pt[:], lhsT, rhs,
                                start=(t == 0), stop=(t == KH * KW - 1),
                            )
                        ot = opool.tile([128, RG, WO], F32, tag="ot")
                        nc.vector.tensor_scalar_add(ot[:], pt[:], bias_t[:, c : c + 1])
                        # Output DMAs ride the scalar (ACT) HWDGE ring so their
                        # sem waits never head-of-line block the input ring.
                        nc.scalar.dma_start(
                            out=o_h[n, c * 128 : (c + 1) * 128, g * RG : (g + 1) * RG, :],
                            in_=ot[:],
                        )
    nc.finalize()
    return nc


_NC_CACHE = None


def _get_nc():
    global _NC_CACHE
    if _NC_CACHE is None:
        _NC_CACHE = build_nc()
    return _NC_CACHE


def _prep_in_maps(x, weight, bias):
    bf16 = ml_dtypes.bfloat16
    # [ci, kh, kw, co] layout so lhsT slices are [ci, co_tile]
    w_t = np.ascontiguousarray(
        weight.astype(np.float32).transpose(1, 2, 3, 0).reshape(CIN, KH * KW * COUT)
    ).astype(bf16)
    b_t = np.ascontiguousarray(bias.astype(np.float32).reshape(COUT, 1))
    in_maps = []
    for i in range(NCORES):
        xs = np.ascontiguousarray(x[i * NPER : (i + 1) * NPER]).astype(bf16)
        in_maps.append({"x": xs, "w": w_t, "b": b_t})
    return in_maps


def run(x, weight, bias, trace=False):
    nc = _get_nc()
    in_maps = _prep_in_maps(x, weight, bias)
    res = run_bass_kernel_spmd(nc, in_maps, core_ids=list(range(NCORES)), trace=trace)
    out = np.concatenate([r["out"] for r in res.results], axis=0)
    return out, res


def kernel(x: np.ndarray, weight: np.ndarray, bias: np.ndarray) -> np.ndarray:
    out, _ = run(x, weight, bias, trace=False)
    return out.astype(np.float32)


if __name__ == "__main__":
    nc = build_nc()
    print("built ok:", len(nc.m.functions[0].blocks if hasattr(nc.m.functions[0], 'blocks') else []), "blocks")
